# revision 2
# baseline (speedup 1.0000x reference)
"""Trainium2 Bass kernel for nn_MemoryBank (vq_codebook softmax) — v3.

C[b, s, t] = softmax_s(2*cross[s,t] - m_sq[s]),  cross = units.T @ H[b]

v3 = v2 + software pipelining. PE does only the cross GEMM (3-term bf16
split), 24 N=512 MMs per (s-chunk, t-half) into PSUM (2 banks x 4 bufs).
The softmax for round r is emitted interleaved into round r+1's s-chunk
loop so every engine FIFO stays busy without blocking PE:
  ACT : evac_k(r+1) [Identity scale=2 bias=-m_sq] alternating exp_k(r)
  DVE : sub_k(r) in-place, den-sum chain, final mul
  GPS : progressive max chain + two partition_all_reduces per round
  1/den via ACT Ln + Exp(scale=-1) to keep DVE's slow RECIPROCAL off
  the critical path. exp values stored bf16 (probs err ~0.4% << 2e-2).
"""
import numpy as np
import ml_dtypes

import concourse.bacc as bacc
import concourse.bass as bass
import concourse.bass_isa as bass_isa
import concourse.mybir as mybir
import concourse.tile as tile
from concourse.tile import add_dep_helper

F32 = mybir.dt.float32
BF16 = mybir.dt.bfloat16
AF = mybir.ActivationFunctionType

B, D, T, S = 64, 512, 2048, 1024
NCORES = 8
B_SH = B // NCORES
DC = D // 128
SBK = S // 128
TH = 1024                   # t half processed per softmax round
NTS = TH // 512


def build_kernel(b_sh=B_SH, t=T, th=TH):
    nh = t // th
    nc = bacc.Bacc(None, target_bir_lowering=False, debug=False)

    h1_d = nc.dram_tensor("h1", [b_sh, DC, 128, t], BF16, kind="ExternalInput")
    h2_d = nc.dram_tensor("h2", [b_sh, DC, 128, t], BF16, kind="ExternalInput")
    u1_d = nc.dram_tensor("u1", [DC, 128, S], BF16, kind="ExternalInput")
    u2_d = nc.dram_tensor("u2", [DC, 128, S], BF16, kind="ExternalInput")
    mq_d = nc.dram_tensor("mq", [128, SBK], F32, kind="ExternalInput")
    c_d = nc.dram_tensor("C", [b_sh, S, t], F32, kind="ExternalOutput")

    with tile.TileContext(nc) as tc:
        with (
            tc.tile_pool(name="const", bufs=1) as cpool,
            tc.tile_pool(name="hbuf", bufs=2) as hpool,
            tc.tile_pool(name="logit", bufs=2) as lpool,
            tc.tile_pool(name="expp", bufs=1) as epool,
            tc.tile_pool(name="stat", bufs=1) as spool,
            tc.tile_pool(name="outp", bufs=3) as opool,
            tc.tile_pool(name="ps", bufs=4, space="PSUM") as ps,
        ):
            mq_sb = cpool.tile([128, SBK], F32, tag="mq")
            nc.sync.dma_start(mq_sb[:], mq_d[:])
            u1_sb = cpool.tile([128, DC, S], BF16, tag="u1")
            u2_sb = cpool.tile([128, DC, S], BF16, tag="u2")

            def load_h(b):
                hb1 = hpool.tile([128, DC, t], BF16, tag="h1", name="h1")
                hb2 = hpool.tile([128, DC, t], BF16, tag="h2", name="h2")
                for c in range(DC):
                    nc.sync.dma_start(hb1[:, c, :], h1_d[b, c])
                for c in range(DC):
                    nc.sync.dma_start(hb2[:, c, :], h2_d[b, c])
                return hb1, hb2

            # startup order: interleave u1/h1(b0) per chunk, then u2/h2,
            # so round 0's matmuls can start as soon as chunk 0 lands
            hb1 = hpool.tile([128, DC, t], BF16, tag="h1", name="h1")
            hb2 = hpool.tile([128, DC, t], BF16, tag="h2", name="h2")
            for c in range(DC):
                nc.sync.dma_start(u1_sb[:, c, :], u1_d[c])
                nc.sync.dma_start(hb1[:, c, :], h1_d[0, c])
            for c in range(DC):
                nc.sync.dma_start(u2_sb[:, c, :], u2_d[c])
                nc.sync.dma_start(hb2[:, c, :], h2_d[0, c])
            h_cur = (hb1, hb2)

            # last t-half processed at shrinking granularity (512+256+256)
            # so the exposed softmax drain at the end stays small
            rounds = [(b, ih * th, th) for b in range(b_sh)
                      for ih in range(nh)][:-1]
            rounds += [(b_sh - 1, t - th, th // 2),
                       (b_sh - 1, t - th // 2, th // 4),
                       (b_sh - 1, t - th // 4, th // 4)]
            prev = None

            def emit_prev_tail(pv):
                """all-reduce den, 1/den via Ln+Exp, muls + output DMA."""
                rh = pv["th"]
                nc.gpsimd.partition_all_reduce(
                    pv["dbc"][:], pv["dsum"][:], channels=128,
                    reduce_op=bass_isa.ReduceOp.add)
                nc.scalar.activation(pv["lnd"][:], pv["dbc"][:], AF.Ln)
                nc.scalar.activation(pv["rden"][:], pv["lnd"][:], AF.Exp,
                                     scale=-1.0)
                for sc in range(SBK):
                    ot = opool.tile([128, rh], F32, tag="ot", name="ot")
                    nc.vector.tensor_mul(ot[:], pv["eps"][sc][:],
                                         pv["rden"][:])
                    nc.sync.dma_start(
                        c_d[pv["b"], sc * 128:(sc + 1) * 128,
                            pv["t0"]:pv["t0"] + rh],
                        ot[:])

            def emit_prev_chunk(pv, sc, after=None):
                """sub + exp + den-accumulate for one s-chunk of round r-1."""
                rh = pv["th"]
                nc.vector.tensor_sub(pv["lps"][sc][:], pv["lps"][sc][:],
                                     pv["mbc"][:])
                ep = epool.tile([128, rh], BF16, tag=f"ep{sc}", name=f"ep{sc}")
                ei = nc.scalar.activation(ep[:], pv["lps"][sc][:], AF.Exp)
                if after is not None:
                    # pin the Scalar-queue order: the current round's PSUM
                    # evacuation must never sit behind this exp, whose
                    # gpsimd-gated input arrives late on real hardware
                    add_dep_helper(ei.ins, after.ins, sync=True,
                                   reason="exp after next-round evac")
                if sc == 0:
                    nc.vector.tensor_copy(pv["dsum"][:], ep[:])
                else:
                    nc.vector.tensor_add(pv["dsum"][:], pv["dsum"][:], ep[:])
                pv["eps"].append(ep)

            h_tiles = {0: h_cur}
            for r, (b, t0, rh) in enumerate(rounds):
                if t0 == 0 and b + 1 < b_sh and (b + 1) not in h_tiles:
                    # prefetch next batch's H one full batch ahead
                    h_tiles[b + 1] = load_h(b + 1)
                h1_sb, h2_sb = h_tiles[b]
                tw = min(rh, 512)
                nts = rh // tw
                tmax = spool.tile([128, rh], F32, tag="tmax", name="tmax")
                lps = []
                for sc in range(SBK):
                    cr = ps.tile([128, rh], F32, tag="cr", name="cr")
                    s0 = sc * 128
                    terms = ((u1_sb, h1_sb), (u2_sb, h1_sb), (u1_sb, h2_sb))
                    order = ([(c, ti) for c in range(DC) for ti in range(3)]
                             if r > 0 else
                             [(c, ti) for ti in range(3) for c in range(DC)])
                    for c, ti in order:
                        uu, hh = terms[ti]
                        if True:
                            for ts in range(nts):
                                nc.tensor.matmul(
                                    cr[:, ts * tw:(ts + 1) * tw],
                                    uu[:, c, s0:s0 + 128],
                                    hh[:, c, t0 + ts * tw:t0 + (ts + 1) * tw],
                                    start=(c == 0 and ti == 0),
                                    stop=(c == DC - 1 and ti == 2),
                                    skip_group_check=True,
                                )
                    lp = lpool.tile([128, rh], F32, tag=f"lp{sc}",
                                    name=f"lp{sc}")
                    ev = nc.scalar.activation(
                        lp[:], cr[:], AF.Identity,
                        bias=mq_sb[:, sc:sc + 1], scale=2.0)
                    if sc == 1:
                        nc.vector.tensor_max(tmax[:], lps[0][:], lp[:])
                    elif sc > 1:
                        nc.vector.tensor_max(tmax[:], tmax[:], lp[:])
                    lps.append(lp)
                    if prev is not None:
                        emit_prev_chunk(prev, sc, after=ev)

                # prev tail first: its all-reduce-add must precede this
                # round's all-reduce-max in the GpSimd FIFO, else Ln/rden
                # (and every later Scalar-queue evac) block behind it
                if prev is not None:
                    emit_prev_tail(prev)

                mbc = spool.tile([128, rh], F32, tag="mbc", name="mbc")
                nc.gpsimd.partition_all_reduce(
                    mbc[:], tmax[:], channels=128,
                    reduce_op=bass_isa.ReduceOp.max)

                prev = {
                    "b": b, "t0": t0, "th": rh, "lps": lps, "mbc": mbc,
                    "eps": [],
                    "dsum": spool.tile([128, rh], BF16, tag="dsum",
                                       name="dsum"),
                    "dbc": spool.tile([128, rh], F32, tag="dbc", name="dbc"),
                    "lnd": spool.tile([128, rh], F32, tag="lnd", name="lnd"),
                    "rden": spool.tile([128, rh], F32, tag="rden",
                                       name="rden"),
                }

            # drain the final round
            for sc in range(SBK):
                emit_prev_chunk(prev, sc)
            emit_prev_tail(prev)

    nc.compile()
    return nc


# ---------------------------------------------------------------- host side

_RUNNER = None


def _get_runner():
    global _RUNNER
    if _RUNNER is None:
        nc = build_kernel()
        _RUNNER = _BassPjrtRunner(nc, NCORES)
    return _RUNNER


def _split_bf16(x):
    hi = x.astype(ml_dtypes.bfloat16)
    lo = (x - hi.astype(np.float32)).astype(ml_dtypes.bfloat16)
    return hi, lo


def prep_inputs(H, units):
    H = np.ascontiguousarray(np.asarray(H, dtype=np.float32))
    U = np.ascontiguousarray(np.asarray(units, dtype=np.float32))
    h1, h2 = _split_bf16(H)
    u1, u2 = _split_bf16(U)
    msqn = (-(U.astype(np.float64) ** 2).sum(0)).astype(np.float32)
    mq = np.ascontiguousarray(msqn.reshape(SBK, 128).T)   # [128, SBK]

    u1 = u1.reshape(DC, 128, S)
    u2 = u2.reshape(DC, 128, S)
    in_maps = []
    for c in range(NCORES):
        sl = slice(c * B_SH, (c + 1) * B_SH)
        in_maps.append({
            "h1": h1[sl].reshape(B_SH, DC, 128, T),
            "h2": h2[sl].reshape(B_SH, DC, 128, T),
            "u1": u1, "u2": u2, "mq": mq,
        })
    return in_maps


def kernel(H, units):
    runner = _get_runner()
    in_maps = prep_inputs(H, units)
    args = runner.prep_inputs(in_maps)
    outs = runner.run(args)
    c = np.asarray(outs[0])
    return c.reshape(B, S, T)


# ------------------------------------------------- embedded PJRT runner

class _BassPjrtRunner:
    def __init__(self, nc, n_cores):
        import jax
        from jax.sharding import Mesh, PartitionSpec
        from jax.experimental.shard_map import shard_map
        from concourse import bass2jax

        bass2jax.install_neuronx_cc_hook()
        self.n_cores = n_cores
        partition_name = (
            nc.partition_id_tensor.name if nc.partition_id_tensor else None
        )
        in_names, out_names, out_avals, zero_outs = [], [], [], []
        for alloc in nc.m.functions[0].allocations:
            if not isinstance(alloc, mybir.MemoryLocationSet):
                continue
            name = alloc.memorylocations[0].name
            if alloc.kind == "ExternalInput":
                if name != partition_name:
                    in_names.append(name)
            elif alloc.kind == "ExternalOutput":
                shape = tuple(alloc.tensor_shape)
                dtype = mybir.dt.np(alloc.dtype)
                out_names.append(name)
                out_avals.append(jax.core.ShapedArray(shape, dtype))
                zero_outs.append((shape, dtype))
        self.in_names = in_names
        self.out_names = out_names
        self.out_shapes = zero_outs
        n_params = len(in_names)
        n_outs = len(out_avals)
        all_in_names = in_names + out_names
        if partition_name is not None:
            all_in_names.append(partition_name)
        self.n_params = n_params

        def _body(*args):
            operands = list(args)
            if partition_name is not None:
                operands.append(bass2jax.partition_id_tensor())
            outs = bass2jax._bass_exec_p.bind(
                *operands,
                out_avals=tuple(out_avals),
                in_names=tuple(all_in_names),
                out_names=tuple(out_names),
                lowering_input_output_aliases=(),
                sim_require_finite=False,
                sim_require_nnan=False,
                nc=nc,
            )
            return tuple(outs)

        devices = jax.devices()[:n_cores]
        assert len(devices) == n_cores
        if n_cores == 1:
            self._fn = jax.jit(_body, keep_unused=True)
        else:
            mesh = Mesh(np.asarray(devices), ("core",))
            in_specs = (PartitionSpec("core"),) * (n_params + n_outs)
            out_specs = (PartitionSpec("core"),) * n_outs
            self._fn = jax.jit(
                shard_map(_body, mesh=mesh, in_specs=in_specs,
                          out_specs=out_specs, check_rep=False),
                keep_unused=True,
            )

    def prep_inputs(self, in_maps):
        per_core = [[np.asarray(m[n]) for n in self.in_names] for m in in_maps]
        if self.n_cores == 1:
            args = per_core[0]
        else:
            args = [
                np.concatenate([per_core[c][i] for c in range(self.n_cores)], 0)
                for i in range(self.n_params)
            ]
        zouts = []
        for (s, d) in self.out_shapes:
            full = (s[0] * self.n_cores,) + tuple(s[1:]) \
                if self.n_cores > 1 else s
            zouts.append(np.zeros(full, d))
        return args + zouts

    def run(self, args):
        import jax
        outs = self._fn(*args)
        jax.block_until_ready(outs)
        return outs


# revision 3
# speedup vs baseline: 1.0126x; 1.0126x over previous
"""Trainium2 Bass kernel for nn_MemoryBank (vq_codebook softmax).

C[b, s, t] = softmax_s(2*cross[s,t] - m_sq[s]),  cross = units.T @ H[b]

Software-pipelined design. PE does only the cross GEMM (3-term bf16
split), 24 N=512 MMs per (s-chunk, t-half) into PSUM (2 banks x 4 bufs).
The softmax for round r is emitted interleaved into round r+1's s-chunk
loop so every engine FIFO stays busy without blocking PE:
  ACT : evac_k(r+1) [Identity scale=2 bias=-m_sq] alternating exp_k(r)
  DVE : sub_k(r) in-place, den-sum chain, final mul
  GPS : progressive max chain + two partition_all_reduces per round
  1/den via ACT Ln + Exp(scale=-1) to keep DVE's slow RECIPROCAL off
  the critical path. exp values stored bf16 (probs err ~0.4% << 2e-2).
"""
import numpy as np
import ml_dtypes

import concourse.bacc as bacc
import concourse.bass as bass
import concourse.bass_isa as bass_isa
import concourse.mybir as mybir
import concourse.tile as tile
from concourse.tile import add_dep_helper

F32 = mybir.dt.float32
BF16 = mybir.dt.bfloat16
AF = mybir.ActivationFunctionType

B, D, T, S = 64, 512, 2048, 1024
NCORES = 8
B_SH = B // NCORES
DC = D // 128
SBK = S // 128
TH = 1024                   # t half processed per softmax round
NTS = TH // 512


def build_kernel(b_sh=B_SH, t=T, th=TH):
    nh = t // th
    nc = bacc.Bacc(None, target_bir_lowering=False, debug=False)

    h1_d = nc.dram_tensor("h1", [b_sh, DC, 128, t], BF16, kind="ExternalInput")
    h2_d = nc.dram_tensor("h2", [b_sh, DC, 128, t], BF16, kind="ExternalInput")
    u1_d = nc.dram_tensor("u1", [DC, 128, S], BF16, kind="ExternalInput")
    u2_d = nc.dram_tensor("u2", [DC, 128, S], BF16, kind="ExternalInput")
    mq_d = nc.dram_tensor("mq", [128, SBK], F32, kind="ExternalInput")
    c_d = nc.dram_tensor("C", [b_sh, S, t], F32, kind="ExternalOutput")

    with tile.TileContext(nc) as tc:
        with (
            tc.tile_pool(name="const", bufs=1) as cpool,
            tc.tile_pool(name="hbuf", bufs=2) as hpool,
            tc.tile_pool(name="logit", bufs=2) as lpool,
            tc.tile_pool(name="expp", bufs=1) as epool,
            tc.tile_pool(name="stat", bufs=1) as spool,
            tc.tile_pool(name="outp", bufs=3) as opool,
            tc.tile_pool(name="ps", bufs=4, space="PSUM") as ps,
        ):
            mq_sb = cpool.tile([128, SBK], F32, tag="mq")
            nc.sync.dma_start(mq_sb[:], mq_d[:])
            u1_sb = cpool.tile([128, DC, S], BF16, tag="u1")
            u2_sb = cpool.tile([128, DC, S], BF16, tag="u2")

            def load_h(b):
                hb1 = hpool.tile([128, DC, t], BF16, tag="h1", name="h1")
                hb2 = hpool.tile([128, DC, t], BF16, tag="h2", name="h2")
                for c in range(DC):
                    nc.sync.dma_start(hb1[:, c, :], h1_d[b, c])
                for c in range(DC):
                    nc.sync.dma_start(hb2[:, c, :], h2_d[b, c])
                return hb1, hb2

            # startup order: interleave u1/h1(b0) per chunk, then u2/h2,
            # so round 0's matmuls can start as soon as chunk 0 lands
            hb1 = hpool.tile([128, DC, t], BF16, tag="h1", name="h1")
            hb2 = hpool.tile([128, DC, t], BF16, tag="h2", name="h2")
            for c in range(DC):
                nc.sync.dma_start(u1_sb[:, c, :], u1_d[c])
                nc.sync.dma_start(hb1[:, c, :], h1_d[0, c])
            for c in range(DC):
                nc.sync.dma_start(u2_sb[:, c, :], u2_d[c])
                nc.sync.dma_start(hb2[:, c, :], h2_d[0, c])
            h_cur = (hb1, hb2)

            # last t-half processed at shrinking granularity (512+256+256)
            # so the exposed softmax drain at the end stays small
            rounds = [(b, ih * th, th) for b in range(b_sh)
                      for ih in range(nh)][:-1]
            rounds += [(b_sh - 1, t - th + q * (th // 2), th // 2)
                       for q in range(2)]
            prev = None

            def emit_prev_tail(pv):
                """all-reduce den, 1/den via Ln+Exp, muls + output DMA."""
                rh = pv["th"]
                nc.gpsimd.partition_all_reduce(
                    pv["dbc"][:], pv["dsum"][:], channels=128,
                    reduce_op=bass_isa.ReduceOp.add)
                nc.scalar.activation(pv["lnd"][:], pv["dbc"][:], AF.Ln)
                nc.scalar.activation(pv["rden"][:], pv["lnd"][:], AF.Exp,
                                     scale=-1.0)
                for sc in range(SBK):
                    ot = opool.tile([128, rh], F32, tag="ot", name="ot")
                    nc.vector.tensor_mul(ot[:], pv["eps"][sc][:],
                                         pv["rden"][:])
                    nc.sync.dma_start(
                        c_d[pv["b"], sc * 128:(sc + 1) * 128,
                            pv["t0"]:pv["t0"] + rh],
                        ot[:])

            def emit_prev_chunk(pv, sc, after=None):
                """sub + exp + den-accumulate for one s-chunk of round r-1."""
                rh = pv["th"]
                nc.vector.tensor_sub(pv["lps"][sc][:], pv["lps"][sc][:],
                                     pv["mbc"][:])
                ep = epool.tile([128, rh], BF16, tag=f"ep{sc}", name=f"ep{sc}")
                ei = nc.scalar.activation(ep[:], pv["lps"][sc][:], AF.Exp)
                if after is not None:
                    # pin the Scalar-queue order: the current round's PSUM
                    # evacuation must never sit behind this exp, whose
                    # gpsimd-gated input arrives late on real hardware
                    add_dep_helper(ei.ins, after.ins, sync=False,
                                   reason="exp after next-round evac")
                if sc == 0:
                    nc.vector.tensor_copy(pv["dsum"][:], ep[:])
                else:
                    nc.vector.tensor_add(pv["dsum"][:], pv["dsum"][:], ep[:])
                pv["eps"].append(ep)

            h_tiles = {0: h_cur}
            for r, (b, t0, rh) in enumerate(rounds):
                if t0 == 0 and b + 1 < b_sh and (b + 1) not in h_tiles:
                    # prefetch next batch's H one full batch ahead
                    h_tiles[b + 1] = load_h(b + 1)
                h1_sb, h2_sb = h_tiles[b]
                tw = min(rh, 512)
                nts = rh // tw
                tmax = spool.tile([128, rh], F32, tag="tmax", name="tmax")
                lps = []
                for sc in range(SBK):
                    cr = ps.tile([128, rh], F32, tag="cr", name="cr")
                    s0 = sc * 128
                    terms = ((u1_sb, h1_sb), (u2_sb, h1_sb), (u1_sb, h2_sb))
                    order = ([(c, ti) for c in range(DC) for ti in range(3)]
                             if r > 0 else
                             [(c, ti) for ti in range(3) for c in range(DC)])
                    for c, ti in order:
                        uu, hh = terms[ti]
                        if True:
                            for ts in range(nts):
                                nc.tensor.matmul(
                                    cr[:, ts * tw:(ts + 1) * tw],
                                    uu[:, c, s0:s0 + 128],
                                    hh[:, c, t0 + ts * tw:t0 + (ts + 1) * tw],
                                    start=(c == 0 and ti == 0),
                                    stop=(c == DC - 1 and ti == 2),
                                    skip_group_check=True,
                                )
                    lp = lpool.tile([128, rh], F32, tag=f"lp{sc}",
                                    name=f"lp{sc}")
                    ev = nc.scalar.activation(
                        lp[:], cr[:], AF.Identity,
                        bias=mq_sb[:, sc:sc + 1], scale=2.0)
                    if sc == 1:
                        nc.vector.tensor_max(tmax[:], lps[0][:], lp[:])
                    elif sc > 1:
                        nc.vector.tensor_max(tmax[:], tmax[:], lp[:])
                    lps.append(lp)
                    if prev is not None:
                        emit_prev_chunk(prev, sc, after=ev)

                # prev tail first: its all-reduce-add must precede this
                # round's all-reduce-max in the GpSimd FIFO, else Ln/rden
                # (and every later Scalar-queue evac) block behind it
                if prev is not None:
                    emit_prev_tail(prev)

                mbc = spool.tile([128, rh], F32, tag="mbc", name="mbc")
                nc.gpsimd.partition_all_reduce(
                    mbc[:], tmax[:], channels=128,
                    reduce_op=bass_isa.ReduceOp.max)

                prev = {
                    "b": b, "t0": t0, "th": rh, "lps": lps, "mbc": mbc,
                    "eps": [],
                    "dsum": spool.tile([128, rh], BF16, tag="dsum",
                                       name="dsum"),
                    "dbc": spool.tile([128, rh], F32, tag="dbc", name="dbc"),
                    "lnd": spool.tile([128, rh], F32, tag="lnd", name="lnd"),
                    "rden": spool.tile([128, rh], F32, tag="rden",
                                       name="rden"),
                }

            # drain the final round
            for sc in range(SBK):
                emit_prev_chunk(prev, sc)
            emit_prev_tail(prev)

    nc.compile()
    return nc


# ---------------------------------------------------------------- host side

_RUNNER = None


def _get_runner():
    global _RUNNER
    if _RUNNER is None:
        nc = build_kernel()
        _RUNNER = _BassPjrtRunner(nc, NCORES)
    return _RUNNER


def _split_bf16(x):
    hi = x.astype(ml_dtypes.bfloat16)
    lo = (x - hi.astype(np.float32)).astype(ml_dtypes.bfloat16)
    return hi, lo


def prep_inputs(H, units):
    H = np.ascontiguousarray(np.asarray(H, dtype=np.float32))
    U = np.ascontiguousarray(np.asarray(units, dtype=np.float32))
    h1, h2 = _split_bf16(H)
    u1, u2 = _split_bf16(U)
    msqn = (-(U.astype(np.float64) ** 2).sum(0)).astype(np.float32)
    mq = np.ascontiguousarray(msqn.reshape(SBK, 128).T)   # [128, SBK]

    u1 = u1.reshape(DC, 128, S)
    u2 = u2.reshape(DC, 128, S)
    in_maps = []
    for c in range(NCORES):
        sl = slice(c * B_SH, (c + 1) * B_SH)
        in_maps.append({
            "h1": h1[sl].reshape(B_SH, DC, 128, T),
            "h2": h2[sl].reshape(B_SH, DC, 128, T),
            "u1": u1, "u2": u2, "mq": mq,
        })
    return in_maps


def kernel(H, units):
    runner = _get_runner()
    in_maps = prep_inputs(H, units)
    args = runner.prep_inputs(in_maps)
    outs = runner.run(args)
    c = np.asarray(outs[0])
    return c.reshape(B, S, T)


# ------------------------------------------------- embedded PJRT runner

class _BassPjrtRunner:
    def __init__(self, nc, n_cores):
        import jax
        from jax.sharding import Mesh, PartitionSpec
        from jax.experimental.shard_map import shard_map
        from concourse import bass2jax

        bass2jax.install_neuronx_cc_hook()
        self.n_cores = n_cores
        partition_name = (
            nc.partition_id_tensor.name if nc.partition_id_tensor else None
        )
        in_names, out_names, out_avals, zero_outs = [], [], [], []
        for alloc in nc.m.functions[0].allocations:
            if not isinstance(alloc, mybir.MemoryLocationSet):
                continue
            name = alloc.memorylocations[0].name
            if alloc.kind == "ExternalInput":
                if name != partition_name:
                    in_names.append(name)
            elif alloc.kind == "ExternalOutput":
                shape = tuple(alloc.tensor_shape)
                dtype = mybir.dt.np(alloc.dtype)
                out_names.append(name)
                out_avals.append(jax.core.ShapedArray(shape, dtype))
                zero_outs.append((shape, dtype))
        self.in_names = in_names
        self.out_names = out_names
        self.out_shapes = zero_outs
        n_params = len(in_names)
        n_outs = len(out_avals)
        all_in_names = in_names + out_names
        if partition_name is not None:
            all_in_names.append(partition_name)
        self.n_params = n_params

        def _body(*args):
            operands = list(args)
            if partition_name is not None:
                operands.append(bass2jax.partition_id_tensor())
            outs = bass2jax._bass_exec_p.bind(
                *operands,
                out_avals=tuple(out_avals),
                in_names=tuple(all_in_names),
                out_names=tuple(out_names),
                lowering_input_output_aliases=(),
                sim_require_finite=False,
                sim_require_nnan=False,
                nc=nc,
            )
            return tuple(outs)

        devices = jax.devices()[:n_cores]
        assert len(devices) == n_cores
        if n_cores == 1:
            self._fn = jax.jit(_body, keep_unused=True)
        else:
            mesh = Mesh(np.asarray(devices), ("core",))
            in_specs = (PartitionSpec("core"),) * (n_params + n_outs)
            out_specs = (PartitionSpec("core"),) * n_outs
            self._fn = jax.jit(
                shard_map(_body, mesh=mesh, in_specs=in_specs,
                          out_specs=out_specs, check_rep=False),
                keep_unused=True,
            )

    def prep_inputs(self, in_maps):
        per_core = [[np.asarray(m[n]) for n in self.in_names] for m in in_maps]
        if self.n_cores == 1:
            args = per_core[0]
        else:
            args = [
                np.concatenate([per_core[c][i] for c in range(self.n_cores)], 0)
                for i in range(self.n_params)
            ]
        zouts = []
        for (s, d) in self.out_shapes:
            full = (s[0] * self.n_cores,) + tuple(s[1:]) \
                if self.n_cores > 1 else s
            zouts.append(np.zeros(full, d))
        return args + zouts

    def run(self, args):
        import jax
        outs = self._fn(*args)
        jax.block_until_ready(outs)
        return outs


# revision 4
# speedup vs baseline: 1.0138x; 1.0012x over previous
"""Trainium2 Bass kernel for nn_MemoryBank (vq_codebook softmax).

C[b, s, t] = softmax_s(2*cross[s,t] - m_sq[s]),  cross = units.T @ H[b]

Software-pipelined design. PE does only the cross GEMM (3-term bf16
split), 24 N=512 MMs per (s-chunk, t-half) into PSUM (2 banks x 4 bufs).
The softmax for round r is emitted interleaved into round r+1's s-chunk
loop so every engine FIFO stays busy without blocking PE:
  ACT : evac_k(r+1) [Identity scale=2 bias=-m_sq] alternating exp_k(r)
  DVE : sub_k(r) in-place, den-sum chain, final mul
  GPS : progressive max chain + two partition_all_reduces per round
  1/den via ACT Ln + Exp(scale=-1) to keep DVE's slow RECIPROCAL off
  the critical path. exp values stored bf16 (probs err ~0.4% << 2e-2).
"""
import numpy as np
import ml_dtypes

import concourse.bacc as bacc
import concourse.bass as bass
import concourse.bass_isa as bass_isa
import concourse.mybir as mybir
import concourse.tile as tile
from concourse.tile import add_dep_helper

F32 = mybir.dt.float32
BF16 = mybir.dt.bfloat16
AF = mybir.ActivationFunctionType

B, D, T, S = 64, 512, 2048, 1024
NCORES = 8
B_SH = B // NCORES
DC = D // 128
SBK = S // 128
TH = 1024                   # t half processed per softmax round
NTS = TH // 512


def build_kernel(b_sh=B_SH, t=T, th=TH):
    nh = t // th
    nc = bacc.Bacc(None, target_bir_lowering=False, debug=False)

    h1_d = nc.dram_tensor("h1", [b_sh, DC, 128, t], BF16, kind="ExternalInput")
    h2_d = nc.dram_tensor("h2", [b_sh, DC, 128, t], BF16, kind="ExternalInput")
    u1_d = nc.dram_tensor("u1", [DC, 128, S], BF16, kind="ExternalInput")
    u2_d = nc.dram_tensor("u2", [DC, 128, S], BF16, kind="ExternalInput")
    mq_d = nc.dram_tensor("mq", [128, SBK], F32, kind="ExternalInput")
    c_d = nc.dram_tensor("C", [b_sh, S, t], F32, kind="ExternalOutput")

    with tile.TileContext(nc) as tc:
        with (
            tc.tile_pool(name="const", bufs=1) as cpool,
            tc.tile_pool(name="hbuf", bufs=2) as hpool,
            tc.tile_pool(name="logit", bufs=2) as lpool,
            tc.tile_pool(name="expp", bufs=1) as epool,
            tc.tile_pool(name="stat", bufs=1) as spool,
            tc.tile_pool(name="outp", bufs=3) as opool,
            tc.tile_pool(name="ps", bufs=4, space="PSUM") as ps,
        ):
            mq_sb = cpool.tile([128, SBK], F32, tag="mq")
            nc.sync.dma_start(mq_sb[:], mq_d[:])
            u1_sb = cpool.tile([128, DC, S], BF16, tag="u1")
            u2_sb = cpool.tile([128, DC, S], BF16, tag="u2")

            def load_h(b):
                hb1 = hpool.tile([128, DC, t], BF16, tag="h1", name="h1")
                hb2 = hpool.tile([128, DC, t], BF16, tag="h2", name="h2")
                for c in range(DC):
                    nc.sync.dma_start(hb1[:, c, :], h1_d[b, c])
                for c in range(DC):
                    nc.sync.dma_start(hb2[:, c, :], h2_d[b, c])
                return hb1, hb2

            # startup order: interleave u1/h1(b0) per chunk, then u2/h2,
            # so round 0's matmuls can start as soon as chunk 0 lands
            hb1 = hpool.tile([128, DC, t], BF16, tag="h1", name="h1")
            hb2 = hpool.tile([128, DC, t], BF16, tag="h2", name="h2")
            # split the first chunks so round 0's first matmuls gate on
            # ~0.4MB instead of ~1.5MB
            nc.sync.dma_start(u1_sb[:, 0, 0:512], u1_d[0][:, 0:512])
            nc.sync.dma_start(hb1[:, 0, 0:1024], h1_d[0, 0][:, 0:1024])
            nc.sync.dma_start(u1_sb[:, 0, 512:1024], u1_d[0][:, 512:1024])
            nc.sync.dma_start(hb1[:, 0, 1024:2048], h1_d[0, 0][:, 1024:2048])
            for c in range(1, DC):
                nc.sync.dma_start(u1_sb[:, c, :], u1_d[c])
                nc.sync.dma_start(hb1[:, c, :], h1_d[0, c])
            # warm the PE clock (HAM) with junk matmuls during the DMA
            # wall: ~24 N=256 MMs end before the first real data lands
            warm = cpool.tile([128, 256], BF16, tag="warm")
            nc.vector.memset(warm[:], 0.0)
            wps = ps.tile([128, 256], F32, tag="cr", name="warmps")
            for _ in range(24):
                nc.tensor.matmul(wps[:], warm[:, 0:128], warm[:, 0:256],
                                 start=True, stop=True,
                                 skip_group_check=True)
            for c in range(DC):
                nc.sync.dma_start(u2_sb[:, c, :], u2_d[c])
                nc.sync.dma_start(hb2[:, c, :], h2_d[0, c])
            h_cur = (hb1, hb2)

            # last t-half processed at shrinking granularity (512+256+256)
            # so the exposed softmax drain at the end stays small
            rounds = [(b, ih * th, th) for b in range(b_sh)
                      for ih in range(nh)][:-1]
            rounds += [(b_sh - 1, t - th + q * (th // 2), th // 2)
                       for q in range(2)]
            prev = None

            def emit_prev_tail(pv):
                """all-reduce den, 1/den via Ln+Exp, muls + output DMA."""
                rh = pv["th"]
                nc.gpsimd.partition_all_reduce(
                    pv["dbc"][:], pv["dsum"][:], channels=128,
                    reduce_op=bass_isa.ReduceOp.add)
                nc.scalar.activation(pv["lnd"][:], pv["dbc"][:], AF.Ln)
                nc.scalar.activation(pv["rden"][:], pv["lnd"][:], AF.Exp,
                                     scale=-1.0)
                for sc in range(SBK):
                    ot = opool.tile([128, rh], F32, tag="ot", name="ot")
                    nc.vector.tensor_mul(ot[:], pv["eps"][sc][:],
                                         pv["rden"][:])
                    nc.sync.dma_start(
                        c_d[pv["b"], sc * 128:(sc + 1) * 128,
                            pv["t0"]:pv["t0"] + rh],
                        ot[:])

            def emit_prev_chunk(pv, sc, after=None):
                """sub + exp + den-accumulate for one s-chunk of round r-1."""
                rh = pv["th"]
                nc.vector.tensor_sub(pv["lps"][sc][:], pv["lps"][sc][:],
                                     pv["mbc"][:])
                ep = epool.tile([128, rh], BF16, tag=f"ep{sc}", name=f"ep{sc}")
                ei = nc.scalar.activation(ep[:], pv["lps"][sc][:], AF.Exp)
                if after is not None:
                    # pin the Scalar-queue order: the current round's PSUM
                    # evacuation must never sit behind this exp, whose
                    # gpsimd-gated input arrives late on real hardware
                    add_dep_helper(ei.ins, after.ins, sync=False,
                                   reason="exp after next-round evac")
                if sc == 0:
                    nc.vector.tensor_copy(pv["dsum"][:], ep[:])
                else:
                    nc.vector.tensor_add(pv["dsum"][:], pv["dsum"][:], ep[:])
                pv["eps"].append(ep)

            h_tiles = {0: h_cur}
            for r, (b, t0, rh) in enumerate(rounds):
                if t0 == 0 and b + 1 < b_sh and (b + 1) not in h_tiles:
                    # prefetch next batch's H one full batch ahead
                    h_tiles[b + 1] = load_h(b + 1)
                h1_sb, h2_sb = h_tiles[b]
                tw = min(rh, 512)
                nts = rh // tw
                tmax = spool.tile([128, rh], F32, tag="tmax", name="tmax")
                lps = []
                for sc in range(SBK):
                    cr = ps.tile([128, rh], F32, tag="cr", name="cr")
                    s0 = sc * 128
                    terms = ((u1_sb, h1_sb), (u2_sb, h1_sb), (u1_sb, h2_sb))
                    order = ([(c, ti) for c in range(DC) for ti in range(3)]
                             if r > 0 else
                             [(c, ti) for ti in range(3) for c in range(DC)])
                    for c, ti in order:
                        uu, hh = terms[ti]
                        if True:
                            for ts in range(nts):
                                nc.tensor.matmul(
                                    cr[:, ts * tw:(ts + 1) * tw],
                                    uu[:, c, s0:s0 + 128],
                                    hh[:, c, t0 + ts * tw:t0 + (ts + 1) * tw],
                                    start=(c == 0 and ti == 0),
                                    stop=(c == DC - 1 and ti == 2),
                                    skip_group_check=True,
                                )
                    lp = lpool.tile([128, rh], F32, tag=f"lp{sc}",
                                    name=f"lp{sc}")
                    ev = nc.scalar.activation(
                        lp[:], cr[:], AF.Identity,
                        bias=mq_sb[:, sc:sc + 1], scale=2.0)
                    if sc == 1:
                        nc.vector.tensor_max(tmax[:], lps[0][:], lp[:])
                    elif sc > 1:
                        nc.vector.tensor_max(tmax[:], tmax[:], lp[:])
                    lps.append(lp)
                    if prev is not None:
                        emit_prev_chunk(prev, sc, after=ev)

                # prev tail first: its all-reduce-add must precede this
                # round's all-reduce-max in the GpSimd FIFO, else Ln/rden
                # (and every later Scalar-queue evac) block behind it
                if prev is not None:
                    emit_prev_tail(prev)

                mbc = spool.tile([128, rh], F32, tag="mbc", name="mbc")
                nc.gpsimd.partition_all_reduce(
                    mbc[:], tmax[:], channels=128,
                    reduce_op=bass_isa.ReduceOp.max)

                prev = {
                    "b": b, "t0": t0, "th": rh, "lps": lps, "mbc": mbc,
                    "eps": [],
                    "dsum": spool.tile([128, rh], BF16, tag="dsum",
                                       name="dsum"),
                    "dbc": spool.tile([128, rh], F32, tag="dbc", name="dbc"),
                    "lnd": spool.tile([128, rh], F32, tag="lnd", name="lnd"),
                    "rden": spool.tile([128, rh], F32, tag="rden",
                                       name="rden"),
                }

            # drain the final round
            for sc in range(SBK):
                emit_prev_chunk(prev, sc)
            emit_prev_tail(prev)

    nc.compile()
    return nc


# ---------------------------------------------------------------- host side

_RUNNER = None


def _get_runner():
    global _RUNNER
    if _RUNNER is None:
        nc = build_kernel()
        _RUNNER = _BassPjrtRunner(nc, NCORES)
    return _RUNNER


def _split_bf16(x):
    hi = x.astype(ml_dtypes.bfloat16)
    lo = (x - hi.astype(np.float32)).astype(ml_dtypes.bfloat16)
    return hi, lo


def prep_inputs(H, units):
    H = np.ascontiguousarray(np.asarray(H, dtype=np.float32))
    U = np.ascontiguousarray(np.asarray(units, dtype=np.float32))
    h1, h2 = _split_bf16(H)
    u1, u2 = _split_bf16(U)
    msqn = (-(U.astype(np.float64) ** 2).sum(0)).astype(np.float32)
    mq = np.ascontiguousarray(msqn.reshape(SBK, 128).T)   # [128, SBK]

    u1 = u1.reshape(DC, 128, S)
    u2 = u2.reshape(DC, 128, S)
    in_maps = []
    for c in range(NCORES):
        sl = slice(c * B_SH, (c + 1) * B_SH)
        in_maps.append({
            "h1": h1[sl].reshape(B_SH, DC, 128, T),
            "h2": h2[sl].reshape(B_SH, DC, 128, T),
            "u1": u1, "u2": u2, "mq": mq,
        })
    return in_maps


def kernel(H, units):
    runner = _get_runner()
    in_maps = prep_inputs(H, units)
    args = runner.prep_inputs(in_maps)
    outs = runner.run(args)
    c = np.asarray(outs[0])
    return c.reshape(B, S, T)


# ------------------------------------------------- embedded PJRT runner

class _BassPjrtRunner:
    def __init__(self, nc, n_cores):
        import jax
        from jax.sharding import Mesh, PartitionSpec
        from jax.experimental.shard_map import shard_map
        from concourse import bass2jax

        bass2jax.install_neuronx_cc_hook()
        self.n_cores = n_cores
        partition_name = (
            nc.partition_id_tensor.name if nc.partition_id_tensor else None
        )
        in_names, out_names, out_avals, zero_outs = [], [], [], []
        for alloc in nc.m.functions[0].allocations:
            if not isinstance(alloc, mybir.MemoryLocationSet):
                continue
            name = alloc.memorylocations[0].name
            if alloc.kind == "ExternalInput":
                if name != partition_name:
                    in_names.append(name)
            elif alloc.kind == "ExternalOutput":
                shape = tuple(alloc.tensor_shape)
                dtype = mybir.dt.np(alloc.dtype)
                out_names.append(name)
                out_avals.append(jax.core.ShapedArray(shape, dtype))
                zero_outs.append((shape, dtype))
        self.in_names = in_names
        self.out_names = out_names
        self.out_shapes = zero_outs
        n_params = len(in_names)
        n_outs = len(out_avals)
        all_in_names = in_names + out_names
        if partition_name is not None:
            all_in_names.append(partition_name)
        self.n_params = n_params

        def _body(*args):
            operands = list(args)
            if partition_name is not None:
                operands.append(bass2jax.partition_id_tensor())
            outs = bass2jax._bass_exec_p.bind(
                *operands,
                out_avals=tuple(out_avals),
                in_names=tuple(all_in_names),
                out_names=tuple(out_names),
                lowering_input_output_aliases=(),
                sim_require_finite=False,
                sim_require_nnan=False,
                nc=nc,
            )
            return tuple(outs)

        devices = jax.devices()[:n_cores]
        assert len(devices) == n_cores
        if n_cores == 1:
            self._fn = jax.jit(_body, keep_unused=True)
        else:
            mesh = Mesh(np.asarray(devices), ("core",))
            in_specs = (PartitionSpec("core"),) * (n_params + n_outs)
            out_specs = (PartitionSpec("core"),) * n_outs
            self._fn = jax.jit(
                shard_map(_body, mesh=mesh, in_specs=in_specs,
                          out_specs=out_specs, check_rep=False),
                keep_unused=True,
            )

    def prep_inputs(self, in_maps):
        per_core = [[np.asarray(m[n]) for n in self.in_names] for m in in_maps]
        if self.n_cores == 1:
            args = per_core[0]
        else:
            args = [
                np.concatenate([per_core[c][i] for c in range(self.n_cores)], 0)
                for i in range(self.n_params)
            ]
        zouts = []
        for (s, d) in self.out_shapes:
            full = (s[0] * self.n_cores,) + tuple(s[1:]) \
                if self.n_cores > 1 else s
            zouts.append(np.zeros(full, d))
        return args + zouts

    def run(self, args):
        import jax
        outs = self._fn(*args)
        jax.block_until_ready(outs)
        return outs
